# revision 1
# baseline (speedup 1.0000x reference)
"""GCN layer (PyG GCNConv equivalent) on 8 Trainium2 NeuronCores.

out[v] = sum_{(u,v) in E + self-loops} dinv[u]*dinv[v]*x[u] @ W + b,
with deg computed at target nodes (including self-loops).

Linearity lets us aggregate raw scaled features first and apply W once:
    xs = x * dinv[:, None]
    agg[v] = sum_e dinv[dst_e] * xs[src_e]      (dinv[dst] carried in a
                                                 per-tile selection matrix)
    out = agg @ W + b

Sharding: destination nodes are partitioned across the 8 cores (12544 per
core, 98 blocks of 128); each core receives the full xs table (replicated)
plus its own edge slots, sorted by destination block and padded to a fixed
T tiles of 128 edges per block (pad edges gather a zero row with weight 0).

Per block on-device:
  - T indirect DMAs gather the block's edge source rows from HBM
    (HW semantics: one int32 row index per output partition).
  - per tile, a fused tensor_scalar builds S[e, v] = sdst[e]*(dstloc[e]==v);
    PSUM accumulates aggT[feat, v] += G_t.T @ S_t over the T tiles.
  - out_block = aggT.T @ W + b  (second matmul + bias add), DMA to y.
"""

import numpy as np

import concourse.bass as bass
import concourse.bacc as bacc
import concourse.tile as tile
import concourse.mybir as mybir
from concourse import bass_utils

P = 128
D = 128
N_CORES = 8


def _build_nc(NB, T, XS_ROWS, num_devices=N_CORES, gather_bufs=3,
              dyn_reps=False):
    f32 = mybir.dt.float32
    i32 = mybir.dt.int32
    NPC = NB * P

    nc = bacc.Bacc("TRN2", target_bir_lowering=False, debug=False,
                   num_devices=num_devices)
    xs_d = nc.dram_tensor("xs", [XS_ROWS, D], f32, kind="ExternalInput").ap()
    srcs_d = nc.dram_tensor("srcs", [P, NB * T], i32, kind="ExternalInput").ap()
    dstloc_d = nc.dram_tensor("dstloc", [P, NB * T], f32,
                              kind="ExternalInput").ap()
    sdst_d = nc.dram_tensor("sdst", [P, NB * T], f32,
                            kind="ExternalInput").ap()
    w_d = nc.dram_tensor("w", [D, D], f32, kind="ExternalInput").ap()
    bb_d = nc.dram_tensor("bb", [P, D], f32, kind="ExternalInput").ap()
    y_d = nc.dram_tensor("y", [NPC, D], f32, kind="ExternalOutput").ap()
    if dyn_reps:
        nreps_d = nc.dram_tensor("nreps", [1, 1], i32,
                                 kind="ExternalInput").ap()

    with tile.TileContext(nc) as tc:
        with (
            tc.tile_pool(name="const", bufs=1) as cpool,
            tc.tile_pool(name="gather", bufs=gather_bufs) as gpool,
            tc.tile_pool(name="sel", bufs=4) as spool,
            tc.tile_pool(name="outsb", bufs=3) as opool,
            tc.tile_pool(name="psum", bufs=2, space="PSUM") as ppool,
        ):
            srcs_sb = cpool.tile([P, NB * T], i32, tag="srcs")
            dstloc_sb = cpool.tile([P, NB * T], f32, tag="dstloc")
            sdst_sb = cpool.tile([P, NB * T], f32, tag="sdst")
            w_sb = cpool.tile([P, D], f32, tag="w")
            bb_sb = cpool.tile([P, D], f32, tag="bb")
            nc.sync.dma_start(out=srcs_sb[:], in_=srcs_d[:])
            nc.sync.dma_start(out=dstloc_sb[:], in_=dstloc_d[:])
            nc.sync.dma_start(out=sdst_sb[:], in_=sdst_d[:])
            nc.sync.dma_start(out=w_sb[:], in_=w_d[:])
            nc.sync.dma_start(out=bb_sb[:], in_=bb_d[:])

            iota_i = cpool.tile([P, P], i32, tag="iota_i")
            iota_f = cpool.tile([P, P], f32, tag="iota_f")
            nc.gpsimd.iota(iota_i[:], pattern=[[1, P]], base=0,
                           channel_multiplier=0)
            nc.vector.tensor_copy(iota_f[:], iota_i[:])

            def body():
                for b in range(NB):
                    g = gpool.tile([P, T * D], f32, tag="g")
                    for t in range(T):
                        col = b * T + t
                        nc.gpsimd.indirect_dma_start(
                            out=g[:, t * D:(t + 1) * D],
                            out_offset=None,
                            in_=xs_d[:],
                            in_offset=bass.IndirectOffsetOnAxis(
                                ap=srcs_sb[:, col:col + 1], axis=0),
                        )
                    aggT_ps = ppool.tile([P, P], f32, tag="agg")
                    for t in range(T):
                        col = b * T + t
                        s = spool.tile([P, P], f32, tag="s")
                        nc.vector.tensor_scalar(
                            s[:], iota_f[:],
                            dstloc_sb[:, col:col + 1],
                            sdst_sb[:, col:col + 1],
                            op0=mybir.AluOpType.is_equal,
                            op1=mybir.AluOpType.mult,
                        )
                        nc.tensor.matmul(
                            aggT_ps[:],
                            lhsT=g[:, t * D:(t + 1) * D],
                            rhs=s[:],
                            start=(t == 0),
                            stop=(t == T - 1),
                        )
                    aggT_sb = opool.tile([P, P], f32, tag="aggsb")
                    nc.vector.tensor_copy(aggT_sb[:], aggT_ps[:])
                    out_ps = ppool.tile([P, P], f32, tag="out")
                    nc.tensor.matmul(out_ps[:], lhsT=aggT_sb[:], rhs=w_sb[:],
                                     start=True, stop=True)
                    y_sb = opool.tile([P, D], f32, tag="ysb")
                    nc.vector.tensor_tensor(y_sb[:], out_ps[:], bb_sb[:],
                                            op=mybir.AluOpType.add)
                    nc.sync.dma_start(out=y_d[b * P:(b + 1) * P, :],
                                      in_=y_sb[:])

            if dyn_reps:
                nr_sb = cpool.tile([1, 1], i32, tag="nr")
                nc.sync.dma_start(out=nr_sb[:], in_=nreps_d[:])
                regs = nc.alloc_registers("nreps_regs")
                nc.regs_load(regs, nr_sb[0:1, 0:1])
                r = nc.snap(regs, donate=True, min_val=1, max_val=10000)
                with tc.For_i(0, r):
                    body()
            else:
                body()

    nc.compile()
    return nc


def _host_prep(x, edge_index, W, b, n_cores=N_CORES):
    N = x.shape[0]
    src = np.asarray(edge_index[0], dtype=np.int64)
    dst = np.asarray(edge_index[1], dtype=np.int64)

    deg = np.bincount(dst, minlength=N).astype(np.float32) + 1.0
    dinv = (1.0 / np.sqrt(deg)).astype(np.float32)
    xs = np.asarray(x, dtype=np.float32) * dinv[:, None]

    loops = np.arange(N, dtype=np.int64)
    src = np.concatenate([src, loops])
    dst = np.concatenate([dst, loops])

    NPC = -(-N // (n_cores * P)) * P
    NB = NPC // P
    ZR = -(-(N + 1) // P) * P          # zero-row index for pad edges
    XS_ROWS = ZR + P
    xs_pad = np.zeros((XS_ROWS, D), dtype=np.float32)
    xs_pad[:N] = xs

    core = dst // NPC
    block = (dst - core * NPC) // P

    cb = core * NB + block
    counts = np.bincount(cb, minlength=n_cores * NB)
    T = max(1, int(-(-counts.max() // P)))

    order = np.argsort(cb, kind="stable")
    src_s = src[order].astype(np.int32)
    dstloc_s = ((dst - core * NPC) % P)[order].astype(np.float32)
    sdst_s = dinv[dst][order].astype(np.float32)
    cb_s = cb[order]

    starts = np.zeros(n_cores * NB, dtype=np.int64)
    starts[1:] = np.cumsum(counts)[:-1]
    within = np.arange(len(cb_s)) - starts[cb_s]

    srcs_pad = np.full((n_cores * NB, T * P), ZR, dtype=np.int32)
    dstloc_pad = np.zeros((n_cores * NB, T * P), dtype=np.float32)
    sdst_pad = np.zeros((n_cores * NB, T * P), dtype=np.float32)
    flat_pos = cb_s * (T * P) + within
    srcs_pad.ravel()[flat_pos] = src_s
    dstloc_pad.ravel()[flat_pos] = dstloc_s
    sdst_pad.ravel()[flat_pos] = sdst_s

    srcs_pad = srcs_pad.reshape(n_cores, NB, T, P)
    dstloc_pad = dstloc_pad.reshape(n_cores, NB, T, P)
    sdst_pad = sdst_pad.reshape(n_cores, NB, T, P)

    Wf = np.ascontiguousarray(np.asarray(W, dtype=np.float32))
    bb = np.ascontiguousarray(
        np.broadcast_to(np.asarray(b, dtype=np.float32), (P, D)))

    in_maps = []
    for c in range(n_cores):
        in_maps.append({
            "xs": xs_pad,
            "srcs": np.ascontiguousarray(
                srcs_pad[c].transpose(2, 0, 1).reshape(P, NB * T)),
            "dstloc": np.ascontiguousarray(
                dstloc_pad[c].transpose(2, 0, 1).reshape(P, NB * T)),
            "sdst": np.ascontiguousarray(
                sdst_pad[c].transpose(2, 0, 1).reshape(P, NB * T)),
            "w": Wf,
            "bb": bb,
        })
    return in_maps, (NB, T, XS_ROWS, NPC)


_NC_CACHE = {}


def _get_nc(meta, dyn_reps=False):
    key = (meta, dyn_reps)
    if key not in _NC_CACHE:
        NB, T, XS_ROWS, NPC = meta
        _NC_CACHE[key] = _build_nc(NB, T, XS_ROWS, dyn_reps=dyn_reps)
    return _NC_CACHE[key]


def kernel(x, edge_index, W, b):
    x = np.asarray(x)
    N = x.shape[0]
    in_maps, meta = _host_prep(x, edge_index, W, b)
    nc = _get_nc(meta)
    res = bass_utils.run_bass_kernel_spmd(
        nc, in_maps, core_ids=list(range(N_CORES)))
    y = np.concatenate([res.results[c]["y"] for c in range(N_CORES)], axis=0)
    return np.ascontiguousarray(y[:N]).astype(np.float32)
